# revision 4
# baseline (speedup 1.0000x reference)
"""Trainium2 Bass kernel for nn_CostFn_18562848653837.

reference(x, cond, time) only reads x[b, j, 6+k] for j in [0,26), k in [0,6)
(~2.6 MB of the 436 MB input; cond/time are unused) and computes, per point,
the reflected mass 1 / (u^T J M^{-1} J^T u) with u = e_x, which reduces via
Sherman-Morrison (M = 2I + 0.5 c c^T) to pure functions of sin^2(cq) and
sin(2*cq), cq = cumsum(q):

    Q1 = sum_k L_k^2 sin^2(cq_k)      Q3 = sum_k sin^2(cq_k)
    P2 = sum_k L_k sin(2 cq_k)        TC = 2.5 - 0.25*Q3
    cost = TC / (0.5*Q1*TC - 0.03125*P2^2)

Host marshalling: both sin^2(th) and sin(2 th) are invariant under
th -> th - k*pi, so the host ships m = cq/pi - rne(cq/pi) in [-0.5, 0.5]
as bf16 (rel tol is 2e-2; bf16 end-to-end error measured at ~1e-4), laid
out k-minor as one (128, 624) tile per core (13312 points x 6 joints).

Device per core (one pass, 8-way batch data parallel):
  - input lands as two 312-col DMA chunks (Pool + SP queues, in parallel);
  - ACT: per chunk, SM = Sin(pi*m) and SF = Sin(2pi~*m) (bf16 out; the
    one-ulp-shaded 2pi keeps the [-pi, pi] table domain);
  - DVE: SMSQ = SM*SM (bf16 2x mode), Q3 via one segmented tensor_reduce,
    reciprocal, and a fused tensor_tensor_reduce producing COST = TC*R
    plus its per-partition row-sum in one instruction;
  - Pool: the L^2- and L-weighted k-sums as 6-step scalar_tensor_tensor
    ladders over k-strided views (weights folded into the scalars - no
    pattern constants), plus the cheap f32 chain ops TC/G/TB/D;
  - output: a dma_scatter_add descriptor is PREPARED at t~0 (idx iota) and
    only TRIGGERED after the last row-sum lands - the trigger path skips
    the ~500ns issue + ~650ns DGE delay of a plain dma_start, leaving just
    transfer + sem propagation (~1.0us instead of ~2.5us of tail).
    Scatter-add requires a 256B row stride, so out is (128, 64) f32 with
    the two chunk partials in columns 0..1; it ADDS into DRAM, so the
    runner must feed a zeroed output buffer (it does - donated zeros).

Host sums the 8 cores' (128, 2) partials in f32.
"""

import numpy as np

_P = 128
_COLS = 624  # 104 points x 6 joints, k-minor
_K = 6
_NCORES = 8
_B, _H, _T = 4096, 1024, 26
_BPC = _B // _NCORES
_SPLIT = 384  # chunk boundary (col index, multiple of 6); chunk 1 kept small
              # so the last ladder/chain tail after the final Sin is short
_OUTW = 64  # 64 f32 = 256B row stride (scatter-add constraint)
_NCHUNK = 2

_CACHE = {}


def _get_nc():
    if "nc" in _CACHE:
        return _CACHE["nc"]

    import concourse.tile as tile
    import concourse.mybir as mybir
    from concourse import bacc

    PI32 = float(np.float32(np.pi))
    # One-ulp-shaded 2*pi: |m| <= 0.5 exactly, so |SCALE2*m| <= pi*(1-2^-23)
    # stays strictly inside the Sin table domain.
    SCALE2 = float(np.float32(2.0 * np.pi * (1.0 - 2.0**-23)))
    L = [float(np.float32(v)) for v in np.arange(1, 7) * 0.1 + 0.3]
    L2 = [v * v for v in L]

    f32 = mybir.dt.float32
    bf16 = mybir.dt.bfloat16
    i16 = mybir.dt.int16
    AX = mybir.AxisListType
    OP = mybir.AluOpType
    ACT = mybir.ActivationFunctionType

    nc = bacc.Bacc(
        "TRN2", target_bir_lowering=False, debug=False, num_devices=_NCORES,
        disable_frame_to_traceback=True,
    )
    q_dram = nc.dram_tensor("q", [_P, _COLS], bf16, kind="ExternalInput")
    out_dram = nc.dram_tensor("out", [_P, _OUTW], f32, kind="ExternalOutput")

    chunks = [(0, _SPLIT), (_SPLIT, _COLS)]

    with (
        tile.TileContext(nc) as tc,
        tc.tile_pool(name="pool", bufs=1) as pool,
    ):
        # ---- t~0 setup: input DMAs + output scatter-add prep --------------
        X = pool.tile([_P, _COLS], bf16)
        # chunk 0 on the Pool queue (arrives first), chunk 1 on SP
        nc.gpsimd.dma_start(X[:, 0:_SPLIT], q_dram[:, 0:_SPLIT])
        nc.sync.dma_start(X[:, _SPLIT:_COLS], q_dram[:, _SPLIT:_COLS])

        # scatter-add indices: token i (partition i) -> out row i.
        # executor reads idxs[c, s] for token c+16s from a 128-partition AP.
        IDX = pool.tile([_P, 8], i16)
        nc.gpsimd.iota(IDX[:], pattern=[[16, 8]], base=0, channel_multiplier=1)
        nc.gpsimd.tensor_scalar_min(IDX[:], IDX[:], _P - 1)
        COL = pool.tile([_P, _OUTW], f32)
        nc.vector.memset(COL[:], 0.0)
        dma_sem = nc.alloc_semaphore()
        prep = nc.gpsimd.dma_scatter_add(
            out_dram[:], COL[:].rearrange("p (t e) -> p t e", t=1), IDX[:],
            num_idxs=_P, num_idxs_reg=_P, elem_size=_OUTW,
            prepare_only=True, sem=dma_sem,
        )

        # ---- per-chunk pipeline ------------------------------------------
        SM = pool.tile([_P, _COLS], bf16)
        SF = pool.tile([_P, _COLS], bf16)
        SMSQ = pool.tile([_P, _COLS], bf16)
        W = pool.tile([_P, _COLS // _K], bf16)
        V = pool.tile([_P, _COLS // _K], bf16)
        U = pool.tile([_P, _COLS // _K], f32)
        TC = pool.tile([_P, _COLS // _K], f32)
        G = pool.tile([_P, _COLS // _K], f32)
        TB = pool.tile([_P, _COLS // _K], f32)
        D = pool.tile([_P, _COLS // _K], f32)
        R = pool.tile([_P, _COLS // _K], f32)
        COSTC = pool.tile([_P, _COLS // _K], f32)

        ttrs = []
        for c, (c0, c1) in enumerate(chunks):
            s = slice(c0, c1)
            p = slice(c0 // _K, c1 // _K)  # point range of this chunk
            # ACT: the two trig passes (bf16 in / bf16 out)
            nc.scalar.activation(SM[:, s], X[:, s], ACT.Sin, scale=PI32)
            nc.scalar.activation(SF[:, s], X[:, s], ACT.Sin, scale=SCALE2)
            # DVE: sin^2 (bf16 2x) and the unweighted k-sum Q3
            nc.vector.tensor_mul(SMSQ[:, s], SM[:, s], SM[:, s])
            nc.vector.reduce_sum(
                U[:, p], SMSQ[:, s].rearrange("p (w k) -> p w k", k=_K),
                axis=AX.X,
            )
            # Pool: weighted k-sum ladders over k-strided views
            SQv = SMSQ[:, s].rearrange("p (w k) -> p k w", k=_K)
            SFv = SF[:, s].rearrange("p (w k) -> p k w", k=_K)
            nc.gpsimd.tensor_scalar_mul(W[:, p], SQv[:, 0], L2[0])
            for k in range(1, _K):
                nc.gpsimd.scalar_tensor_tensor(
                    W[:, p], SQv[:, k], L2[k], W[:, p], OP.mult, OP.add
                )
            nc.gpsimd.tensor_scalar_mul(V[:, p], SFv[:, 0], L[0])
            for k in range(1, _K):
                nc.gpsimd.scalar_tensor_tensor(
                    V[:, p], SFv[:, k], L[k], V[:, p], OP.mult, OP.add
                )
            # chain: TC = 2.5 - 0.25*U; G = 0.5*W*TC; TB = 0.03125*V^2;
            # D = G - TB; R = 1/D; COST = TC*R (+ row-sum into COL[:, c]).
            # TC/G/D on DVE (U lands there; Pool stays free for ladders),
            # TB on Pool right after its V ladder.
            nc.vector.tensor_scalar(
                TC[:, p], U[:, p], -0.25, 2.5, OP.mult, OP.add
            )
            nc.vector.scalar_tensor_tensor(
                G[:, p], W[:, p], 0.5, TC[:, p], OP.mult, OP.mult
            )
            nc.gpsimd.scalar_tensor_tensor(
                TB[:, p], V[:, p], 0.03125, V[:, p], OP.mult, OP.mult
            )
            nc.vector.tensor_sub(D[:, p], G[:, p], TB[:, p])
            nc.vector.reciprocal(R[:, p], D[:, p])
            ttr = nc.vector.tensor_tensor_reduce(
                COSTC[:, p], TC[:, p], R[:, p], 1.0, 0.0,
                OP.mult, OP.add, COL[:, c : c + 1],
            )
            ttrs.append(ttr)

        # ---- triggered output --------------------------------------------
        trig = nc.gpsimd.trigger_dma(None)
        for ttr in ttrs:
            tile.add_dep_helper(trig.ins, ttr.ins, sync=True, reason="COL ready")
        wt = nc.gpsimd.wait_ge(dma_sem, 16)
        tile.add_dep_helper(wt.ins, trig.ins, sync=False, reason="after trigger")

    nc.compile()
    _CACHE["nc"] = nc
    return nc


def _shard(x):
    """(B, H, T) f32 -> (8, 128, 624) bf16 of range-reduced cq/pi."""
    import ml_dtypes

    qs = np.ascontiguousarray(x[:, :_T, 6 : 6 + _K]).astype(np.float32)
    cq = np.cumsum(qs, axis=-1) * np.float32(1.0 / np.pi)
    m = cq - np.rint(cq)
    return m.astype(ml_dtypes.bfloat16).reshape(_NCORES, _P, _COLS)


def _get_runner():
    """Build the jitted 8-core shard_map executable once (mirrors
    bass2jax.run_bass_via_pjrt's multi-core path) so repeat kernel() calls
    skip retracing/recompiling."""
    if "run" in _CACHE:
        return _CACHE["run"]
    import jax
    from jax.sharding import Mesh, PartitionSpec
    from jax.experimental.shard_map import shard_map
    from concourse import bass2jax

    nc = _get_nc()
    bass2jax.install_neuronx_cc_hook()
    assert nc.dbg_addr is None
    pid_name = nc.partition_id_tensor.name if nc.partition_id_tensor else None
    in_names = ("q", "out") + ((pid_name,) if pid_name else ())

    out_aval = jax.core.ShapedArray((_P, _OUTW), np.float32)

    def _body(q, out_zero):
        operands = [q, out_zero]
        if pid_name is not None:
            operands.append(bass2jax.partition_id_tensor())
        (out,) = bass2jax._bass_exec_p.bind(
            *operands,
            out_avals=(out_aval,),
            in_names=in_names,
            out_names=("out",),
            lowering_input_output_aliases=(),
            sim_require_finite=True,
            sim_require_nnan=True,
            nc=nc,
        )
        return (out,)

    devices = jax.devices()[:_NCORES]
    mesh = Mesh(np.asarray(devices), ("core",))
    sharded = jax.jit(
        shard_map(
            _body,
            mesh=mesh,
            in_specs=(PartitionSpec("core"),) * 2,
            out_specs=(PartitionSpec("core"),),
            check_rep=False,
        ),
        donate_argnums=(1,),
        keep_unused=True,
    )

    def run(planes):
        concat_q = planes.reshape(_NCORES * _P, _COLS)
        zeros = np.zeros((_NCORES * _P, _OUTW), np.float32)
        (out,) = sharded(concat_q, zeros)
        return np.asarray(out).reshape(_NCORES, _P, _OUTW)

    _CACHE["run"] = run
    return run


def _run_library(planes):
    from concourse.bass_utils import run_bass_kernel_spmd

    res = run_bass_kernel_spmd(
        _get_nc(),
        [
            {"q": planes[i], "out": np.zeros((_P, _OUTW), np.float32)}
            for i in range(_NCORES)
        ],
        list(range(_NCORES)),
    )
    return np.stack([r["out"] for r in res.results]).astype(np.float32)


def _run_subprocess(planes):
    """Last resort: the accelerator occasionally reports
    NRT_EXEC_UNIT_UNRECOVERABLE; a fresh process reliably recovers it."""
    import os
    import subprocess
    import sys
    import tempfile

    d = tempfile.mkdtemp()
    inp = os.path.join(d, "planes.npy")
    out = os.path.join(d, "out.npy")
    np.save(inp, planes)
    here = os.path.dirname(os.path.abspath(__file__))
    script = (
        "import sys, numpy as np\n"
        f"sys.path.insert(0, {here!r})\n"
        "import kernel as K\n"
        f"planes = np.load({inp!r})\n"
        "out = K._get_runner()(planes)\n"
        f"np.save({out!r}, out)\n"
    )
    err = None
    for _ in range(2):
        try:
            subprocess.run(
                [sys.executable, "-c", script], check=True, timeout=900,
                stdout=subprocess.DEVNULL, stderr=subprocess.DEVNULL,
            )
            return np.load(out)
        except Exception as e:  # retry once; device usually recovers
            err = e
    raise err


def kernel(x, cond, time):
    x = np.asarray(x)
    planes = _shard(x)
    try:
        partials = _get_runner()(planes)
    except Exception:
        try:
            partials = _run_library(planes)
        except Exception:
            partials = _run_subprocess(planes)
    return np.float32(
        np.asarray(partials)[:, :, :_NCHUNK].sum(dtype=np.float32)
    )


# revision 8
# speedup vs baseline: 1.1054x; 1.1054x over previous
"""Trainium2 Bass kernel for nn_CostFn_18562848653837.

reference(x, cond, time) only reads x[b, j, 6+k] for j in [0,26), k in [0,6)
(~2.6 MB of the 436 MB input; cond/time are unused) and computes, per point,
the reflected mass 1 / (u^T J M^{-1} J^T u) with u = e_x, which reduces via
Sherman-Morrison (M = 2I + 0.5 c c^T) to functions of the cumulative angles
cq = cumsum(q).  With C_k = cos(2 cq_k), S_k = sin(2 cq_k):

    SWC = sum_k L_k^2 C_k     SC = sum_k C_k      V = sum_k L_k S_k
    TC  = 1.75 + 0.125*SC     Q1h = 0.6775 - 0.25*SWC
    cost = TC / (Q1h*TC - 0.03125*V^2)

Host marshalling: sin(2 th) and cos(2 th) are invariant under
th -> th - j*pi, so the host ships TWO bf16 angle tiles per core (tol is
2e-2; bf16 end-to-end error measured ~2e-5): m = cq/pi - rne(cq/pi) in
[-0.5, 0.5] so Sin(2pi~*m) = sin(2 cq), and mc = wrap(m + 0.25) so
Sin(2pi~*mc) = cos(2 cq) - the pi/2 shift rides the same Sin table.
Layout is k-minor, (128, 624) per tile (13312 points x 6 joints per core).

Device per core (one pass, 8-way batch data parallel):
  - four DMA slices (mc then m on each of the Pool/SP queues) so the
    first Cos chunk lands right as the ~1.3us Sin table load finishes;
  - ACT: 4 Sin passes, order pinned C0, S0, C1, S1 (big chunk first, the
    short V-tail chunk last);
  - Pool: the L^2- and L-weighted k-sums as 6-step scalar_tensor_tensor
    ladders over k-strided views (weights folded into scalars - no
    pattern constants, no separate multiplies), plus TC/Q1h/G/TB/D;
  - DVE: SC via one segmented tensor_reduce per chunk, the reciprocal,
    and a fused tensor_tensor_reduce emitting COST = TC*R plus its
    per-partition row-sum in one instruction;
  - output: a dma_scatter_add descriptor is PREPARED at t~0 (idx iota)
    and only TRIGGERED after the last row-sum lands - the trigger path
    skips the ~500ns issue + ~650ns DGE delay of a plain dma_start,
    leaving transfer + sem propagation (~1.0us instead of ~2.5us tail).
    Scatter-add needs a 256B row stride, so out is (128, 64) f32 with
    the per-chunk partials in columns 0..1; it ADDS into DRAM, so every
    runner path feeds an explicitly zeroed output buffer.

Host sums the 8 cores' (128, 2) partials in f32.
"""

import numpy as np

_P = 128
_COLS = 624  # 104 points x 6 joints, k-minor
_K = 6
_NCORES = 8
_B, _H, _T = 4096, 1024, 26
_BPC = _B // _NCORES
_SPLIT = 420  # chunk boundary (multiple of 6); chunk 1 kept small so the
              # final ladder/chain tail after the last Sin pass is short
_OUTW = 64  # 64 f32 = 256B row stride (scatter-add constraint)
_NCHUNK = 2

_CACHE = {}


def _get_nc():
    if "nc" in _CACHE:
        return _CACHE["nc"]

    import concourse.tile as tile
    import concourse.mybir as mybir
    from concourse import bacc

    # One-ulp-shaded 2*pi: |m| <= 0.5 exactly, so |SCALE2*m| <= pi*(1-2^-23)
    # stays strictly inside the Sin table domain.
    SCALE2 = float(np.float32(2.0 * np.pi * (1.0 - 2.0**-23)))
    L = [float(np.float32(v)) for v in np.arange(1, 7) * 0.1 + 0.3]
    L2 = [v * v for v in L]

    f32 = mybir.dt.float32
    bf16 = mybir.dt.bfloat16
    i16 = mybir.dt.int16
    AX = mybir.AxisListType
    OP = mybir.AluOpType
    ACT = mybir.ActivationFunctionType

    nc = bacc.Bacc(
        "TRN2", target_bir_lowering=False, debug=False, num_devices=_NCORES,
        disable_frame_to_traceback=True,
    )
    # cols 0:624 = m (for sin 2cq), 624:1248 = mc (for cos 2cq)
    q_dram = nc.dram_tensor("q", [_P, 2 * _COLS], bf16, kind="ExternalInput")
    out_dram = nc.dram_tensor("out", [_P, _OUTW], f32, kind="ExternalOutput")

    chunks = [(0, _SPLIT), (_SPLIT, _COLS)]

    with (
        tile.TileContext(nc) as tc,
        tc.tile_pool(name="pool", bufs=1) as pool,
    ):
        # ---- t~0 setup: input DMAs + output scatter-add prep --------------
        XS = pool.tile([_P, _COLS], bf16)  # m
        XC = pool.tile([_P, _COLS], bf16)  # mc
        # mc first on each queue: the C-passes run first on ACT
        nc.gpsimd.dma_start(XC[:, 0:_SPLIT], q_dram[:, _COLS : _COLS + _SPLIT])
        nc.sync.dma_start(XC[:, _SPLIT:_COLS], q_dram[:, _COLS + _SPLIT :])
        nc.gpsimd.dma_start(XS[:, 0:_SPLIT], q_dram[:, 0:_SPLIT])
        nc.sync.dma_start(XS[:, _SPLIT:_COLS], q_dram[:, _SPLIT:_COLS])

        # scatter-add indices: token i (partition i) -> out row i.
        # executor reads idxs[c, s] for token c+16s from a 128-partition AP.
        IDX = pool.tile([_P, 8], i16)
        nc.gpsimd.iota(IDX[:], pattern=[[16, 8]], base=0, channel_multiplier=1)
        nc.gpsimd.tensor_scalar_min(IDX[:], IDX[:], _P - 1)
        COL = pool.tile([_P, _OUTW], f32)
        nc.vector.memset(COL[:], 0.0)
        dma_sem = nc.alloc_semaphore()
        prep = nc.gpsimd.dma_scatter_add(
            out_dram[:], COL[:].rearrange("p (t e) -> p t e", t=1), IDX[:],
            num_idxs=_P, num_idxs_reg=_P, elem_size=_OUTW,
            prepare_only=True, sem=dma_sem,
        )

        # ---- per-chunk pipeline ------------------------------------------
        C = pool.tile([_P, _COLS], bf16)
        S = pool.tile([_P, _COLS], bf16)
        NPT = _COLS // _K
        SWC = pool.tile([_P, NPT], bf16)
        V = pool.tile([_P, NPT], bf16)
        SC = pool.tile([_P, NPT], f32)
        TC = pool.tile([_P, NPT], f32)
        Q1H = pool.tile([_P, NPT], f32)
        G = pool.tile([_P, NPT], f32)
        TB = pool.tile([_P, NPT], f32)
        D = pool.tile([_P, NPT], f32)
        R = pool.tile([_P, NPT], f32)
        COSTC = pool.tile([_P, NPT], f32)

        ttrs = []
        act_insts = []
        for c, (c0, c1) in enumerate(chunks):
            s = slice(c0, c1)
            p = slice(c0 // _K, c1 // _K)  # point range of this chunk
            ci = nc.scalar.activation(C[:, s], XC[:, s], ACT.Sin, scale=SCALE2)
            si = nc.scalar.activation(S[:, s], XS[:, s], ACT.Sin, scale=SCALE2)
            act_insts += [ci, si]
            # DVE: unweighted k-sum of C (f32 accumulate)
            nc.vector.reduce_sum(
                SC[:, p], C[:, s].rearrange("p (w k) -> p w k", k=_K),
                axis=AX.X,
            )
            # Pool: weighted k-sum ladders over k-strided views
            Cv = C[:, s].rearrange("p (w k) -> p k w", k=_K)
            Sv = S[:, s].rearrange("p (w k) -> p k w", k=_K)
            nc.gpsimd.tensor_scalar_mul(SWC[:, p], Cv[:, 0], L2[0])
            for k in range(1, _K):
                nc.gpsimd.scalar_tensor_tensor(
                    SWC[:, p], Cv[:, k], L2[k], SWC[:, p], OP.mult, OP.add
                )
            nc.gpsimd.tensor_scalar_mul(V[:, p], Sv[:, 0], L[0])
            for k in range(1, _K):
                nc.gpsimd.scalar_tensor_tensor(
                    V[:, p], Sv[:, k], L[k], V[:, p], OP.mult, OP.add
                )
            # chain: TC = 1.75+0.125*SC; Q1h = 0.6775-0.25*SWC; G = Q1h*TC;
            # TB = 0.03125*V^2; D = G-TB; R = 1/D; COST = TC*R (+ row-sum)
            nc.gpsimd.tensor_scalar(
                TC[:, p], SC[:, p], 0.125, 1.75, OP.mult, OP.add
            )
            nc.gpsimd.tensor_scalar(
                Q1H[:, p], SWC[:, p], -0.25, 0.6775, OP.mult, OP.add
            )
            nc.gpsimd.tensor_mul(G[:, p], Q1H[:, p], TC[:, p])
            nc.gpsimd.scalar_tensor_tensor(
                TB[:, p], V[:, p], 0.03125, V[:, p], OP.mult, OP.mult
            )
            nc.gpsimd.tensor_sub(D[:, p], G[:, p], TB[:, p])
            nc.vector.reciprocal(R[:, p], D[:, p])
            ttr = nc.vector.tensor_tensor_reduce(
                COSTC[:, p], TC[:, p], R[:, p], 1.0, 0.0,
                OP.mult, OP.add, COL[:, c : c + 1],
            )
            ttrs.append(ttr)

        # pin ACT pass order: C0, S0, C1, S1 - the last pass feeds only the
        # short V-ladder tail of the small chunk
        for a, b in zip(act_insts, act_insts[1:]):
            tile.add_dep_helper(b.ins, a.ins, sync=False, reason="ACT order")

        # ---- triggered output --------------------------------------------
        trig = nc.gpsimd.trigger_dma(None)
        for ttr in ttrs:
            tile.add_dep_helper(trig.ins, ttr.ins, sync=True, reason="COL ready")
        wt = nc.gpsimd.wait_ge(dma_sem, 16)
        tile.add_dep_helper(wt.ins, trig.ins, sync=False, reason="after trigger")

    nc.compile()
    _CACHE["nc"] = nc
    return nc


def _shard(x):
    """(B, H, T) f32 -> (8, 128, 1248) bf16: [m | mc] with m = frac-reduced
    cq/pi and mc = wrap(m + 1/4) (so device Sin gives sin/cos of 2cq)."""
    import ml_dtypes

    qs = np.ascontiguousarray(x[:, :_T, 6 : 6 + _K]).astype(np.float32)
    cq = np.cumsum(qs, axis=-1) * np.float32(1.0 / np.pi)
    m = cq - np.rint(cq)
    mc = m + np.float32(0.25)
    mc -= (mc > 0.5).astype(np.float32)
    out = np.empty((_NCORES, _P, 2 * _COLS), dtype=ml_dtypes.bfloat16)
    out[:, :, :_COLS] = m.astype(ml_dtypes.bfloat16).reshape(
        _NCORES, _P, _COLS
    )
    out[:, :, _COLS:] = mc.astype(ml_dtypes.bfloat16).reshape(
        _NCORES, _P, _COLS
    )
    return out


def _get_runner():
    """Build the jitted 8-core shard_map executable once (mirrors
    bass2jax.run_bass_via_pjrt's multi-core path) so repeat kernel() calls
    skip retracing/recompiling."""
    if "run" in _CACHE:
        return _CACHE["run"]
    import jax
    from jax.sharding import Mesh, PartitionSpec
    from jax.experimental.shard_map import shard_map
    from concourse import bass2jax

    nc = _get_nc()
    bass2jax.install_neuronx_cc_hook()
    assert nc.dbg_addr is None
    pid_name = nc.partition_id_tensor.name if nc.partition_id_tensor else None
    in_names = ("q", "out") + ((pid_name,) if pid_name else ())

    out_aval = jax.core.ShapedArray((_P, _OUTW), np.float32)

    def _body(q, out_zero):
        operands = [q, out_zero]
        if pid_name is not None:
            operands.append(bass2jax.partition_id_tensor())
        (out,) = bass2jax._bass_exec_p.bind(
            *operands,
            out_avals=(out_aval,),
            in_names=in_names,
            out_names=("out",),
            lowering_input_output_aliases=(),
            sim_require_finite=True,
            sim_require_nnan=True,
            nc=nc,
        )
        return (out,)

    devices = jax.devices()[:_NCORES]
    mesh = Mesh(np.asarray(devices), ("core",))
    sharded = jax.jit(
        shard_map(
            _body,
            mesh=mesh,
            in_specs=(PartitionSpec("core"),) * 2,
            out_specs=(PartitionSpec("core"),),
            check_rep=False,
        ),
        donate_argnums=(1,),
        keep_unused=True,
    )

    def run(planes):
        concat_q = planes.reshape(_NCORES * _P, 2 * _COLS)
        zeros = np.zeros((_NCORES * _P, _OUTW), np.float32)
        (out,) = sharded(concat_q, zeros)
        return np.asarray(out).reshape(_NCORES, _P, _OUTW)

    _CACHE["run"] = run
    return run


def _run_library(planes):
    from concourse.bass_utils import run_bass_kernel_spmd

    res = run_bass_kernel_spmd(
        _get_nc(),
        [
            {"q": planes[i], "out": np.zeros((_P, _OUTW), np.float32)}
            for i in range(_NCORES)
        ],
        list(range(_NCORES)),
    )
    return np.stack([r["out"] for r in res.results]).astype(np.float32)


def _run_subprocess(planes):
    """Last resort: the accelerator occasionally reports
    NRT_EXEC_UNIT_UNRECOVERABLE; a fresh process reliably recovers it."""
    import os
    import subprocess
    import sys
    import tempfile

    d = tempfile.mkdtemp()
    inp = os.path.join(d, "planes.npy")
    out = os.path.join(d, "out.npy")
    np.save(inp, planes)
    here = os.path.dirname(os.path.abspath(__file__))
    script = (
        "import sys, numpy as np, ml_dtypes\n"
        f"sys.path.insert(0, {here!r})\n"
        "import kernel as K\n"
        f"planes = np.load({inp!r})\n"
        "out = K._get_runner()(planes)\n"
        f"np.save({out!r}, out)\n"
    )
    err = None
    for _ in range(2):
        try:
            subprocess.run(
                [sys.executable, "-c", script], check=True, timeout=900,
                stdout=subprocess.DEVNULL, stderr=subprocess.DEVNULL,
            )
            return np.load(out)
        except Exception as e:  # retry once; device usually recovers
            err = e
    raise err


def kernel(x, cond, time):
    x = np.asarray(x)
    planes = _shard(x)
    try:
        partials = _get_runner()(planes)
    except Exception:
        try:
            partials = _run_library(planes)
        except Exception:
            partials = _run_subprocess(planes)
    return np.float32(
        np.asarray(partials)[:, :, :_NCHUNK].sum(dtype=np.float32)
    )


# revision 10
# speedup vs baseline: 1.1846x; 1.0717x over previous
"""Trainium2 Bass kernel for nn_CostFn_18562848653837.

reference(x, cond, time) only reads x[b, j, 6+k] for j in [0,26), k in [0,6)
(~2.6 MB of the 436 MB input; cond/time are unused) and computes, per point,
the reflected mass 1 / (u^T J M^{-1} J^T u) with u = e_x, which reduces via
Sherman-Morrison (M = 2I + 0.5 c c^T) to functions of the cumulative angles
cq = cumsum(q).  With C_k = cos(2 cq_k), S_k = sin(2 cq_k):

    SWC = sum_k L_k^2 C_k     SC = sum_k C_k      V = sum_k L_k S_k
    TC  = 1.75 + 0.125*SC     Q1h = 0.6775 - 0.25*SWC
    cost = TC / (Q1h*TC - 0.03125*V^2)

Host marshalling: sin(2 th) and cos(2 th) are invariant under
th -> th - j*pi, so the host ships TWO bf16 angle tiles per core (tol is
2e-2; bf16 end-to-end error measured ~2e-5): m = cq/pi - rne(cq/pi) in
[-0.5, 0.5] so Sin(2pi~*m) = sin(2 cq), and mc = wrap(m + 0.25) so
Sin(2pi~*mc) = cos(2 cq) - the pi/2 shift rides the same Sin table.
Layout is k-minor, (128, 624) per tile (13312 points x 6 joints per core).

Device per core (one pass, 8-way batch data parallel):
  - four DMA slices (mc then m on each of the Pool/SP queues) so the
    first Cos chunk lands right as the ~1.3us Sin table load finishes;
  - ACT: 4 Sin passes, order pinned C0, S0, C1, S1 (big chunk first, the
    short V-tail chunk last);
  - Pool: the L^2- and L-weighted k-sums as 6-step scalar_tensor_tensor
    ladders over k-strided views (weights folded into scalars - no
    pattern constants, no separate multiplies), plus TC/Q1h/G/TB/D;
  - DVE: SC via one segmented tensor_reduce per chunk, the reciprocal,
    and a fused tensor_tensor_reduce emitting COST = TC*R plus its
    per-partition row-sum in one instruction;
  - output: a dma_scatter_add descriptor is PREPARED at t~0 (idx iota)
    and only TRIGGERED after the last row-sum lands - the trigger path
    skips the ~500ns issue + ~650ns DGE delay of a plain dma_start,
    leaving transfer + sem propagation (~1.0us instead of ~2.5us tail).
    Scatter-add needs a 256B row stride, so out is (128, 64) f32 with
    the per-chunk partials in columns 0..1; it ADDS into DRAM, so every
    runner path feeds an explicitly zeroed output buffer.

Host sums the 8 cores' (128, 2) partials in f32.
"""

import numpy as np

_P = 128
_COLS = 624  # 104 points x 6 joints, k-minor
_K = 6
_NCORES = 8
_B, _H, _T = 4096, 1024, 26
_BPC = _B // _NCORES
_SPLIT = 420  # chunk boundary (multiple of 6); chunk 1 kept small so the
              # final ladder/chain tail after the last Sin pass is short
_OUTW = 64  # 64 f32 = 256B row stride (scatter-add constraint)
_NCHUNK = 2

_CACHE = {}


def _get_nc():
    if "nc" in _CACHE:
        return _CACHE["nc"]

    import concourse.tile as tile
    import concourse.mybir as mybir
    from concourse import bacc

    # One-ulp-shaded 2*pi: |m| <= 0.5 exactly, so |SCALE2*m| <= pi*(1-2^-23)
    # stays strictly inside the Sin table domain.
    SCALE2 = float(np.float32(2.0 * np.pi * (1.0 - 2.0**-23)))
    L = [float(np.float32(v)) for v in np.arange(1, 7) * 0.1 + 0.3]
    L2 = [v * v for v in L]

    f32 = mybir.dt.float32
    bf16 = mybir.dt.bfloat16
    i16 = mybir.dt.int16
    AX = mybir.AxisListType
    OP = mybir.AluOpType
    ACT = mybir.ActivationFunctionType

    nc = bacc.Bacc(
        "TRN2", target_bir_lowering=False, debug=False, num_devices=_NCORES,
        disable_frame_to_traceback=True,
    )
    # cols 0:624 = m (for sin 2cq), 624:1248 = mc (for cos 2cq)
    q_dram = nc.dram_tensor("q", [_P, 2 * _COLS], bf16, kind="ExternalInput")
    out_dram = nc.dram_tensor("out", [_P, _OUTW], f32, kind="ExternalOutput")

    chunks = [(0, _SPLIT), (_SPLIT, _COLS)]

    with (
        tile.TileContext(nc) as tc,
        tc.tile_pool(name="pool", bufs=1) as pool,
    ):
        # ---- t~0 setup: input DMAs + output scatter-add prep --------------
        XS = pool.tile([_P, _COLS], bf16)  # m
        XC = pool.tile([_P, _COLS], bf16)  # mc
        # mc first on each queue: the C-passes run first on ACT
        nc.gpsimd.dma_start(XC[:, 0:_SPLIT], q_dram[:, _COLS : _COLS + _SPLIT])
        nc.sync.dma_start(XC[:, _SPLIT:_COLS], q_dram[:, _COLS + _SPLIT :])
        nc.gpsimd.dma_start(XS[:, 0:_SPLIT], q_dram[:, 0:_SPLIT])
        nc.sync.dma_start(XS[:, _SPLIT:_COLS], q_dram[:, _SPLIT:_COLS])

        # scatter-add indices: token i (partition i) -> out row i.
        # executor reads idxs[c, s] for token c+16s from a 128-partition AP.
        IDX = pool.tile([_P, 8], i16)
        nc.gpsimd.iota(IDX[:], pattern=[[16, 8]], base=0, channel_multiplier=1)
        nc.gpsimd.tensor_scalar_min(IDX[:], IDX[:], _P - 1)
        COL = pool.tile([_P, _OUTW], f32)
        nc.vector.memset(COL[:], 0.0)
        dma_sem = nc.alloc_semaphore()
        prep = nc.gpsimd.dma_scatter_add(
            out_dram[:], COL[:].rearrange("p (t e) -> p t e", t=1), IDX[:],
            num_idxs=_P, num_idxs_reg=_P, elem_size=_OUTW,
            prepare_only=True, sem=dma_sem,
        )

        # ---- per-chunk pipeline ------------------------------------------
        C = pool.tile([_P, _COLS], bf16)
        S = pool.tile([_P, _COLS], bf16)
        NPT = _COLS // _K
        SWC = pool.tile([_P, NPT], bf16)
        V = pool.tile([_P, NPT], bf16)
        SC = pool.tile([_P, NPT], f32)
        TC = pool.tile([_P, NPT], f32)
        Q1H = pool.tile([_P, NPT], f32)
        G = pool.tile([_P, NPT], f32)
        TB = pool.tile([_P, NPT], f32)
        D = pool.tile([_P, NPT], f32)
        R = pool.tile([_P, NPT], f32)
        COSTC = pool.tile([_P, NPT], f32)

        # Engine programs, fully order-pinned (the list scheduler otherwise
        # interleaves chunk-0 leftovers into the critical chunk-1 tail).
        # ACT: C0, S0, C1, S1.  DVE: SC0, chain0's TC/Q1h/G (fills the idle
        # window between the reduces), SC1, recip0, ttr0, recip1, ttr1.
        # Pool: SWC0, V0, SWC1, TB0, D0, V1, chain1, with prep right after
        # the idx setup so it never delays the end-of-kernel trigger.
        act_seq, dve_seq, pool_seq = [], [], []
        pt = [slice(c0 // _K, c1 // _K) for c0, c1 in chunks]

        def lad(out, view, wgt):
            ops = [nc.gpsimd.tensor_scalar_mul(out, view[:, 0], wgt[0])]
            for k in range(1, _K):
                ops.append(
                    nc.gpsimd.scalar_tensor_tensor(
                        out, view[:, k], wgt[k], out, OP.mult, OP.add
                    )
                )
            return ops

        ttrs = []
        for c, (c0, c1) in enumerate(chunks):
            s = slice(c0, c1)
            p = pt[c]
            act_seq.append(
                nc.scalar.activation(C[:, s], XC[:, s], ACT.Sin, scale=SCALE2)
            )
            act_seq.append(
                nc.scalar.activation(S[:, s], XS[:, s], ACT.Sin, scale=SCALE2)
            )

        for c, (c0, c1) in enumerate(chunks):
            s = slice(c0, c1)
            p = pt[c]
            Cv = C[:, s].rearrange("p (w k) -> p k w", k=_K)
            Sv = S[:, s].rearrange("p (w k) -> p k w", k=_K)
            sc_i = nc.vector.reduce_sum(
                SC[:, p], C[:, s].rearrange("p (w k) -> p w k", k=_K),
                axis=AX.X,
            )
            swc_ops = lad(SWC[:, p], Cv, L2)
            v_ops = lad(V[:, p], Sv, L)
            if c == 0:
                # chain0 on DVE (its SC already lives there); TB0/D0 on Pool
                dve_seq += [sc_i]
                dve_seq.append(nc.vector.tensor_scalar(
                    TC[:, p], SC[:, p], 0.125, 1.75, OP.mult, OP.add))
                dve_seq.append(nc.vector.tensor_scalar(
                    Q1H[:, p], SWC[:, p], -0.25, 0.6775, OP.mult, OP.add))
                dve_seq.append(
                    nc.vector.tensor_mul(G[:, p], Q1H[:, p], TC[:, p]))
                pool_seq += swc_ops + v_ops
                pool_seq.append(nc.gpsimd.scalar_tensor_tensor(
                    TB[:, p], V[:, p], 0.03125, V[:, p], OP.mult, OP.mult))
                pool_seq.append(
                    nc.gpsimd.tensor_sub(D[:, p], G[:, p], TB[:, p]))
            else:
                # chunk1: everything between S1 and the reciprocal on Pool
                dve_seq += [sc_i]
                pool_seq += swc_ops
                pool_seq += v_ops
                pool_seq.append(nc.gpsimd.tensor_scalar(
                    TC[:, p], SC[:, p], 0.125, 1.75, OP.mult, OP.add))
                pool_seq.append(nc.gpsimd.tensor_scalar(
                    Q1H[:, p], SWC[:, p], -0.25, 0.6775, OP.mult, OP.add))
                pool_seq.append(
                    nc.gpsimd.tensor_mul(G[:, p], Q1H[:, p], TC[:, p]))
                pool_seq.append(nc.gpsimd.scalar_tensor_tensor(
                    TB[:, p], V[:, p], 0.03125, V[:, p], OP.mult, OP.mult))
                pool_seq.append(
                    nc.gpsimd.tensor_sub(D[:, p], G[:, p], TB[:, p]))
            rc = nc.vector.reciprocal(R[:, p], D[:, p])
            ttr = nc.vector.tensor_tensor_reduce(
                COSTC[:, p], TC[:, p], R[:, p], 1.0, 0.0,
                OP.mult, OP.add, COL[:, c : c + 1],
            )
            if c == 0:
                # recip0/ttr0 slot in after SC1's reduce on the DVE queue
                _pending0 = (rc, ttr)
            else:
                dve_seq += [rc, ttr]
            ttrs.append(ttr)
        # DVE final order: ..., SC1, recip0, ttr0, recip1, ttr1
        dve_seq = dve_seq[:-2] + list(_pending0) + dve_seq[-2:]

        for seq, nm in ((act_seq, "ACT"), (dve_seq, "DVE"), (pool_seq, "PL")):
            for a, b in zip(seq, seq[1:]):
                tile.add_dep_helper(b.ins, a.ins, sync=False, reason=f"{nm} order")

        # ---- triggered output --------------------------------------------
        trig = nc.gpsimd.trigger_dma(None)
        for ttr in ttrs:
            tile.add_dep_helper(trig.ins, ttr.ins, sync=True, reason="COL ready")
        tile.add_dep_helper(trig.ins, pool_seq[-1].ins, sync=False, reason="PL tail")
        wt = nc.gpsimd.wait_ge(dma_sem, 16)
        tile.add_dep_helper(wt.ins, trig.ins, sync=False, reason="after trigger")

    nc.compile()
    _CACHE["nc"] = nc
    return nc


def _shard(x):
    """(B, H, T) f32 -> (8, 128, 1248) bf16: [m | mc] with m = frac-reduced
    cq/pi and mc = wrap(m + 1/4) (so device Sin gives sin/cos of 2cq)."""
    import ml_dtypes

    qs = np.ascontiguousarray(x[:, :_T, 6 : 6 + _K]).astype(np.float32)
    cq = np.cumsum(qs, axis=-1) * np.float32(1.0 / np.pi)
    m = cq - np.rint(cq)
    mc = m + np.float32(0.25)
    mc -= (mc > 0.5).astype(np.float32)
    out = np.empty((_NCORES, _P, 2 * _COLS), dtype=ml_dtypes.bfloat16)
    out[:, :, :_COLS] = m.astype(ml_dtypes.bfloat16).reshape(
        _NCORES, _P, _COLS
    )
    out[:, :, _COLS:] = mc.astype(ml_dtypes.bfloat16).reshape(
        _NCORES, _P, _COLS
    )
    return out


def _get_runner():
    """Build the jitted 8-core shard_map executable once (mirrors
    bass2jax.run_bass_via_pjrt's multi-core path) so repeat kernel() calls
    skip retracing/recompiling."""
    if "run" in _CACHE:
        return _CACHE["run"]
    import jax
    from jax.sharding import Mesh, PartitionSpec
    from jax.experimental.shard_map import shard_map
    from concourse import bass2jax

    nc = _get_nc()
    bass2jax.install_neuronx_cc_hook()
    assert nc.dbg_addr is None
    pid_name = nc.partition_id_tensor.name if nc.partition_id_tensor else None
    in_names = ("q", "out") + ((pid_name,) if pid_name else ())

    out_aval = jax.core.ShapedArray((_P, _OUTW), np.float32)

    def _body(q, out_zero):
        operands = [q, out_zero]
        if pid_name is not None:
            operands.append(bass2jax.partition_id_tensor())
        (out,) = bass2jax._bass_exec_p.bind(
            *operands,
            out_avals=(out_aval,),
            in_names=in_names,
            out_names=("out",),
            lowering_input_output_aliases=(),
            sim_require_finite=True,
            sim_require_nnan=True,
            nc=nc,
        )
        return (out,)

    devices = jax.devices()[:_NCORES]
    mesh = Mesh(np.asarray(devices), ("core",))
    sharded = jax.jit(
        shard_map(
            _body,
            mesh=mesh,
            in_specs=(PartitionSpec("core"),) * 2,
            out_specs=(PartitionSpec("core"),),
            check_rep=False,
        ),
        donate_argnums=(1,),
        keep_unused=True,
    )

    def run(planes):
        concat_q = planes.reshape(_NCORES * _P, 2 * _COLS)
        zeros = np.zeros((_NCORES * _P, _OUTW), np.float32)
        (out,) = sharded(concat_q, zeros)
        return np.asarray(out).reshape(_NCORES, _P, _OUTW)

    _CACHE["run"] = run
    return run


def _run_library(planes):
    from concourse.bass_utils import run_bass_kernel_spmd

    res = run_bass_kernel_spmd(
        _get_nc(),
        [
            {"q": planes[i], "out": np.zeros((_P, _OUTW), np.float32)}
            for i in range(_NCORES)
        ],
        list(range(_NCORES)),
    )
    return np.stack([r["out"] for r in res.results]).astype(np.float32)


def _run_subprocess(planes):
    """Last resort: the accelerator occasionally reports
    NRT_EXEC_UNIT_UNRECOVERABLE; a fresh process reliably recovers it."""
    import os
    import subprocess
    import sys
    import tempfile

    d = tempfile.mkdtemp()
    inp = os.path.join(d, "planes.npy")
    out = os.path.join(d, "out.npy")
    np.save(inp, planes)
    here = os.path.dirname(os.path.abspath(__file__))
    script = (
        "import sys, numpy as np, ml_dtypes\n"
        f"sys.path.insert(0, {here!r})\n"
        "import kernel as K\n"
        f"planes = np.load({inp!r})\n"
        "out = K._get_runner()(planes)\n"
        f"np.save({out!r}, out)\n"
    )
    err = None
    for _ in range(2):
        try:
            subprocess.run(
                [sys.executable, "-c", script], check=True, timeout=900,
                stdout=subprocess.DEVNULL, stderr=subprocess.DEVNULL,
            )
            return np.load(out)
        except Exception as e:  # retry once; device usually recovers
            err = e
    raise err


def kernel(x, cond, time):
    x = np.asarray(x)
    planes = _shard(x)
    try:
        partials = _get_runner()(planes)
    except Exception:
        try:
            partials = _run_library(planes)
        except Exception:
            partials = _run_subprocess(planes)
    return np.float32(
        np.asarray(partials)[:, :, :_NCHUNK].sum(dtype=np.float32)
    )
